# revision 37
# baseline (speedup 1.0000x reference)
"""Trainium2 Bass kernel for ConsistentSelfAttentionTile.

Reference semantics: T=449 overlapping 64-token tiles; each tile attends to
352 KV tokens = 288 sampled (from a 9x replication of the tile) + the tile
itself; outputs overlap-add, then divide by overlap counts.

Algebraic collapse (verified vs the jax reference):
  * rep[:, idx, :] == tile[:, idx % 64, :], so the sampled KV tokens are tile
    rows with integer multiplicities m_t[w] = 1 + #{s : idx[t,s] % 64 == w}.
  * Per-tile Q/K/V are slices of the full-sequence projections, so all
    per-tile 64x64 score blocks are diagonal blocks of one banded 512x512
    score matrix S (band |i-j| <= 63).
  * S itself collapses: S = x G x^T + ones (x) g with G = Wq^T Wk and
    g = bq Wk, both host-precomputed (weight-only folding). Wk/Wq never ship
    to the device and the K projection disappears. bk only shifts rows
    (softmax-invariant): drop.
  * The device computes the TRANSPOSED projection directly:
        QG^T[c',r] = sum_c G[c,c'] x^T[c,r]  (+ g[c'] per-partition bias)
    via 16 128-col matmuls with G row-chunks as lhsT, so no PE transpose of
    QG and only ONE PSUM->SBUF hop on the score path. The g bias is folded
    in during that hop (Identity activation with a per-partition bias AP).
  * ST = x_band QG^T gives exp(ST) = E^T directly; no E transpose needed.
  * No rowmax subtraction: |S| <= ~50, inside fp32/bf16 exponent range; a
    row-constant shift cancels exactly in E/Z, so dropping the max is exact.
  * With E = exp(S), Cm[j,t] = m_t[j-t] (banded):
        Z = Cm^T E^T;  W = maskw * (1/Z);  U = Cm W;  out = (E^T o U)^T V
    maskw bakes in the valid-tile mask and the 1/counts overlap division.
  * bv is folded into the output PSUM as a rank-1 ones (x) bv matmul.
  * Cm^T is transposed on device (PE transpose) instead of shipped.

Sharding: 8 cores = 2 batches x 4 row-chunks of 128 output rows. Each core
computes its 128 rows end-to-end from a 256-column band of the input (no
cross-core communication).

Schedule: the input streams as 9 time-staggered pieces over the two HWDGE
queues (+ gpsimd SWDGE for one Wv chunk), ordered so the score chain
(x^T, G) lands first, cm/mask next, Wv last; consumers chase the stream.
512-col matmuls are SBUF-read-bound (~0.83ns/col) while 128-col chains
sustain ~2x that, so V and the output are emitted as 128-col pieces.
tile_wait_until hints pin the Tile scheduler to estimated arrival times.
The output is computed and evacuated in column halves over both queues.
"""

import os
import sys

import numpy as np

try:
    import ml_dtypes
except ImportError:
    ml_dtypes = None

for _p in ("/opt/trn_rl_repo",):
    if _p not in sys.path and os.path.isdir(_p):
        sys.path.insert(0, _p)

B, N, C, W = 2, 512, 512, 64
T = N - W + 1          # 449 tiles
RCH = 128              # output rows per core
NCORES = 8
BAND = 256             # per-core j/t band width (columns [r0-64, r0+192))
KC = C // 128          # 4 contraction chunks
JC = BAND // 128       # 2 band chunks

# blob16 layout (2-byte elements per partition), grouped by DMA piece.
# B queue (scalar/Act): XT | G2 | G3 | WV2
# A queue (sync/SP):    G0 | G1 | WV0 | CMBLOB (gcols idb cm mw) | WV1
# gpsimd SWDGE:         WV3
OFF_XT = 0                       # [128, 4, 256] fp16  x^T band chunks
OFF_G2 = OFF_XT + KC * BAND      # [128, 512] fp16   G row-chunk 2
OFF_G3 = OFF_G2 + C              # [128, 512]      (B piece 1 = XT|G2|G3)
OFF_G0 = OFF_G3 + C              # [128, 512]      (A piece 1 = G0|G1)
OFF_G1 = OFF_G0 + C              # [128, 512]
OFF_GCOL = OFF_G1 + C            # [128, 8] fp32 bits (A piece 2 = cm blob)
OFF_IDB = OFF_GCOL + 8           # [128, 128] bf16 identity (bitcast)
OFF_CM = OFF_IDB + 128           # [128, 2, 256] bf16 (count ints: exact)
OFF_MW = OFF_CM + JC * BAND      # [128, 2, 128] fp16 mask/counts
OFF_WV0 = OFF_MW + JC * RCH      # [128, 512]      (A piece 3 = WV0|WV1)
OFF_WV1 = OFF_WV0 + C            # [128, 512]
OFF_WV2 = OFF_WV1 + C            # [128, 512]      (pool piece = WV2|WV3)
OFF_WV3 = OFF_WV2 + C            # [128, 512]
F16 = OFF_WV3 + C

# misc row blob [1, 1152] fp16: bv | ones | g
MISC_BV = 0
MISC_ONES = C
MISC_G = C + 128
F_MISC = 2 * C + 128

N_WARM = 5             # dummy matmuls to ungate the PE clock
DEBUG_DUMPS = False    # dump intermediates to DRAM (debug builds only)

_CACHE = {}


def _slim_drain_and_barrier(self, tick_clock, wait_clock):
    """Cheaper TileContext exit: drain only the DMA-queue clocks, one
    sem-only barrier, then reset the Tile semaphores."""
    from concourse.vector_clock import ScopedClock, VectorClock
    from concourse.tile_scheduler import dmasw_start_idx, N_PROCS

    g = tick_clock.global_clock
    dma_clock = VectorClock()
    for idx in range(dmasw_start_idx, N_PROCS):
        t = g.peek_next(idx) - 1
        if t > 0:
            dma_clock.require_at_least(idx, t)
    drain_inst = self.nc.sync.drain()
    wait_clock.add_sem_waits(drain_inst.ins, ScopedClock({None: dma_clock}))
    self.nc.all_engine_barrier(sem_only=True)
    popped = self.nc._tile_sem_poison_stack.pop()
    assert popped is self._sem_poison
    self.nc.clear_and_free_semaphores(list(self.sems.allocated().values()))


def _build_program():
    import concourse.bacc as bacc
    import concourse.mybir as mybir
    import concourse.tile as tile

    fp16 = mybir.dt.float16
    # Skip Bass's preamble all-engine barrier: all real cross-engine deps
    # here are Tile semaphores, and the preamble const APs we read are
    # memset long before their consumers fire.
    orig_aeb = bacc.Bacc.all_engine_barrier

    def _noop_aeb(self, *, sem_only=False):
        return None

    bacc.Bacc.all_engine_barrier = _noop_aeb
    try:
        nc = bacc.Bacc("TRN2", target_bir_lowering=False, debug=False)
    finally:
        bacc.Bacc.all_engine_barrier = orig_aeb

    b16_d = nc.declare_dram_parameter("blob16", [128, F16], fp16, isOutput=False)
    misc_d = nc.declare_dram_parameter("misc", [1, F_MISC], fp16, isOutput=False)
    # output ships as fp16 (host upcasts): halves the final DMA + drain
    out_d = nc.declare_dram_parameter("out", [RCH, C], fp16, isOutput=True)
    dbg_d = None
    if DEBUG_DUMPS:
        bf16 = mybir.dt.bfloat16
        fp32 = mybir.dt.float32
        dbg_d = {
            "dbg_qgt": nc.declare_dram_parameter(
                "dbg_qgt", [128, KC * RCH], fp16, isOutput=True),
            "dbg_et": nc.declare_dram_parameter(
                "dbg_et", [128, JC * RCH], bf16, isOutput=True),
            "dbg_z": nc.declare_dram_parameter(
                "dbg_z", [128, JC * RCH], fp32, isOutput=True),
            "dbg_w": nc.declare_dram_parameter(
                "dbg_w", [128, JC * RCH], bf16, isOutput=True),
            "dbg_a": nc.declare_dram_parameter(
                "dbg_a", [128, JC * RCH], fp16, isOutput=True),
            "dbg_v": nc.declare_dram_parameter(
                "dbg_v", [128, JC * C], fp16, isOutput=True),
            "dbg_cmt": nc.declare_dram_parameter(
                "dbg_cmt", [128, JC * BAND], bf16, isOutput=True),
            "dbg_b16": nc.declare_dram_parameter(
                "dbg_b16", [128, F16], fp16, isOutput=True),
        }

    orig_dab = tile.TileContext._drain_and_barrier
    tile.TileContext._drain_and_barrier = _slim_drain_and_barrier
    try:
        _emit_body(nc, tile, mybir, b16_d, misc_d, out_d, dbg_d)
    finally:
        tile.TileContext._drain_and_barrier = orig_dab

    nc.compile()
    return nc


def _emit_body(nc, tile, mybir, b16_d, misc_d, out_d, dbg_d=None):
    fp32 = mybir.dt.float32
    fp16 = mybir.dt.float16
    bf16 = mybir.dt.bfloat16
    COPY = mybir.ActivationFunctionType.Copy
    IDENT = mybir.ActivationFunctionType.Identity

    with tile.TileContext(nc) as tc:

        def at(us):
            """Schedule hint: don't start the instructions in this block
            before `us` microseconds (relative to kernel-body start)."""
            return tc.tile_wait_until(us / 1000.0)

        with (
            tc.tile_pool(name="consts", bufs=1) as consts,
            tc.tile_pool(name="work", bufs=1) as work,
            tc.tile_pool(name="psum", bufs=1, space="PSUM") as psum,
        ):
            b16 = consts.tile([128, F16], fp16)
            misc = consts.tile([1, F_MISC], fp16)
            warm = work.tile([128, 512], bf16)

            # PE warm-up operand: zeros so nothing downstream can see junk.
            nc.gpsimd.memset(warm[:, :], 0)
            nc.gpsimd.dma_start(out=misc[:, :], in_=misc_d[:, :])

            # ---- input DMA: 2 HWDGE queues x 2 chained pieces + pool ----
            from concourse.tile_rust import add_dep_helper

            def piece(eng, c0, c1, after=None):
                h = eng.dma_start(out=b16[:, c0:c1], in_=b16_d[:, c0:c1])
                if after is not None:
                    add_dep_helper(h.ins, after.ins, True, "input DMA chain")
                return h

            # B (scalar): score stream first, then wv2|wv3
            b1 = piece(nc.scalar, OFF_XT, OFF_G0)            # XT|G2|G3
            # A (sync): G0|G1|gcol, then wv0|wv1
            a1 = piece(nc.sync, OFF_G0, OFF_IDB)
            b2 = piece(nc.scalar, OFF_WV2, F16, after=b1)    # WV2|WV3
            a2 = piece(nc.sync, OFF_WV0, OFF_WV2, after=a1)  # WV0|WV1
            # pool SWDGE: idb|cm|mw in parallel with the HW queues
            piece(nc.gpsimd, OFF_IDB, OFF_WV0)

            # ---- SBUF views ----
            xt_sb = b16[:, OFF_XT:OFF_XT + KC * BAND].rearrange(
                "p (k j) -> p k j", k=KC)
            g_chunk = [b16[:, OFF_G0:OFF_G0 + C], b16[:, OFF_G1:OFF_G1 + C],
                       b16[:, OFF_G2:OFF_G2 + C], b16[:, OFF_G3:OFF_G3 + C]]
            wv_chunk = [b16[:, OFF_WV0:OFF_WV0 + C], b16[:, OFF_WV1:OFF_WV1 + C],
                        b16[:, OFF_WV2:OFF_WV2 + C], b16[:, OFF_WV3:OFF_WV3 + C]]
            gcol = b16[:, OFF_GCOL:OFF_GCOL + 8].bitcast(fp32)
            idb = b16[:, OFF_IDB:OFF_IDB + 128].bitcast(bf16)
            cm_sb = b16[:, OFF_CM:OFF_CM + JC * BAND].bitcast(bf16).rearrange(
                "p (k t) -> p k t", k=JC)
            mw_sb = b16[:, OFF_MW:OFF_MW + JC * RCH].rearrange(
                "p (k r) -> p k r", k=JC)
            bv_row = misc[0:1, MISC_BV:MISC_BV + C]
            ones1 = misc[0:1, MISC_ONES:MISC_ONES + 128]
            g_row = misc[0:1, MISC_G:MISC_G + C]

            # ---- PE clock-gate warm-up (runs while the DMA lands) ----
            ps_aux = psum.tile([128, 512], fp32, tag="ps_aux", bufs=1)
            for _ in range(N_WARM):
                nc.tensor.matmul(
                    ps_aux, lhsT=warm[:, 0:128], rhs=warm[:, :],
                    start=True, stop=True,
                )

            # ---- V[jc][j 128, c 512] = x Wv, 512-col matmuls chasing the
            # Wv stream (wv2|wv3 ride the pool queue and land first)
            v_sb = work.tile([128, JC, C], fp16)
            ps_v0 = psum.tile([128, C], fp32, tag="ps_v0", bufs=1)
            ps_v1 = psum.tile([128, C], fp32, tag="ps_v1", bufs=1)
            ps_vjc = (ps_v0, ps_v1)
            wv_at = {0: 5.6, 1: 7.25, 2: 8.45, 3: 9.15}

            def v_block(cc, start, stop):
                with at(wv_at[cc]):
                    for jc in range(JC):
                        nc.tensor.matmul(
                            ps_vjc[jc],
                            lhsT=xt_sb[:, cc, jc * 128:(jc + 1) * 128],
                            rhs=wv_chunk[cc],
                            start=start,
                            stop=stop,
                        )

            # ---- QG^T[c',r] = sum_cc G[cc,c'] xt[cc,r] + g[c'] (x) ones ----
            # PSUM banks allow only ONE open accumulation group each, so the
            # 4 c'-tiles run as two bank-pairs: (m0 in bank A, m2 in bank B)
            # chunk-chase the G stream, then (m1, m3) reread the resident
            # chunks. The g bias rank-1 closes each group.
            ps_qgtA = psum.tile([128, C], fp32, tag="ps_qgtA", bufs=1)
            ps_qgtB = psum.tile([128, C], fp32, tag="ps_qgtB", bufs=1)
            # pass 2 lands in the (not-yet-used) ST and Z banks so it does
            # not WAR-wait on pass 1's PSUM->SBUF copies
            ps_qgt2a = psum.tile([128, JC, RCH], fp32, tag="ps_st", bufs=1)
            ps_qgt2b = psum.tile([128, JC, RCH], fp32, tag="ps_z", bufs=1)
            ps_of = {0: ps_qgtA[:, 0:RCH], 2: ps_qgtB[:, 0:RCH],
                     1: ps_qgt2a[:, 0, :], 3: ps_qgt2b[:, 0, :]}
            qgt_sb = work.tile([128, KC, RCH], fp16)
            for pi, pair in enumerate(((0, 2), (1, 3))):
                with at(4.5 + 0.85 * pi):
                    for i, cc in enumerate((0, 1, 2, 3)):
                        for m in pair:
                            nc.tensor.matmul(
                                ps_of[m],
                                lhsT=g_chunk[cc][:, m * 128:(m + 1) * 128],
                                rhs=xt_sb[:, cc, 64:64 + RCH],
                                start=(i == 0),
                                stop=(i == KC - 1),
                            )
                # PSUM->SBUF with the g bias folded as a per-partition scalar
                with at(5.4 + 0.85 * pi):
                    for m in pair:
                        nc.vector.tensor_scalar_add(
                            qgt_sb[:, m, :], ps_of[m], gcol[:, m:m + 1])

            v_block(0, True, False)     # wv0|wv1 land ~5.6

            # ---- transposed scores ST[j 256, r 128] and E^T = exp(ST) ----
            ps_st = psum.tile([128, JC, RCH], fp32, tag="ps_st", bufs=1)
            with at(6.85):
                for jc in range(JC):
                    for k in range(KC):
                        nc.tensor.matmul(
                            ps_st[:, jc, :],
                            lhsT=xt_sb[:, k, jc * 128:(jc + 1) * 128],
                            rhs=qgt_sb[:, k, :],
                            start=(k == 0),
                            stop=(k == KC - 1),
                        )
            et_sb = work.tile([128, JC, RCH], bf16)
            with at(7.55):
                nc.scalar.activation(
                    out=et_sb[:, 0, :], in_=ps_st[:, 0, :],
                    func=mybir.ActivationFunctionType.Exp,
                    bias=0.0, scale=1.0,
                )
            with at(7.8):
                nc.scalar.activation(
                    out=et_sb[:, 1, :], in_=ps_st[:, 1, :],
                    func=mybir.ActivationFunctionType.Exp,
                    bias=0.0, scale=1.0,
                )

            # Cm^T on device: 4 PE transposes in the PE gap while exps run
            ps_cmt = psum.tile([128, JC, BAND], bf16, tag="ps_aux", bufs=1)
            with at(7.85):
                for tch in range(JC):
                    for jc in range(JC):
                        nc.tensor.transpose(
                            ps_cmt[:, tch, jc * 128:(jc + 1) * 128],
                            cm_sb[:, jc, tch * 128:(tch + 1) * 128],
                            idb,
                        )
            cmt_sb = work.tile([128, JC, BAND], bf16)
            with at(8.3):
                nc.vector.tensor_copy(out=cmt_sb, in_=ps_cmt)

            v_block(1, False, False)    # wv1 landed with wv0

            # Z[t 128, r 128] per tch = sum_j Cm[j,t] E^T[j,r]
            ps_z = psum.tile([128, JC, RCH], fp32, tag="ps_z", bufs=1)
            with at(8.25):
                for tch in range(JC):
                    for jc in range(JC):
                        nc.tensor.matmul(
                            ps_z[:, tch, :],
                            lhsT=cm_sb[:, jc, tch * 128:(tch + 1) * 128],
                            rhs=et_sb[:, jc, :],
                            start=(jc == 0),
                            stop=(jc == JC - 1),
                        )

            # W[t, r] = maskw / Z: reciprocal on Vector, mask-mul on Pool
            rz_sb = work.tile([128, JC, RCH], fp32)
            w_sb = work.tile([128, JC, RCH], bf16)
            with at(8.6):
                nc.vector.reciprocal_approx_fast(out=rz_sb, in_=ps_z)
            with at(9.0):
                nc.gpsimd.tensor_mul(w_sb, rz_sb, mw_sb)

            v_block(2, False, False)    # wv2|wv3 land ~7.2
            v_block(3, False, True)

            # U[j 128, r 128] per jc = sum_t Cm^T[t,j] W[t,r]
            ps_u = psum.tile([128, JC, RCH], fp32, tag="ps_qgtA", bufs=1)
            with at(9.85):
                for jc in range(JC):
                    for tch in range(JC):
                        nc.tensor.matmul(
                            ps_u[:, jc, :],
                            lhsT=cmt_sb[:, tch, jc * 128:(jc + 1) * 128],
                            rhs=w_sb[:, tch, :],
                            start=(tch == 0),
                            stop=(tch == JC - 1),
                        )
            a_sb = work.tile([128, JC, RCH], fp16)
            with at(10.15):
                nc.vector.tensor_mul(a_sb, ps_u, et_sb)

            # V PSUM -> SBUF in column halves (scalar), feeding out halves
            with at(10.0):
                nc.scalar.activation(out=v_sb[:, 0, 0:256],
                                     in_=ps_v0[:, 0:256], func=COPY)
                nc.scalar.activation(out=v_sb[:, 1, 0:256],
                                     in_=ps_v1[:, 0:256], func=COPY)
            with at(10.7):
                nc.scalar.activation(out=v_sb[:, 0, 256:512],
                                     in_=ps_v0[:, 256:512], func=COPY)
                nc.scalar.activation(out=v_sb[:, 1, 256:512],
                                     in_=ps_v1[:, 256:512], func=COPY)

            # out[r 128, c 512] = sum_j A[j,r] V[j,c] + ones (x) bv,
            # 256-col halves, sequential accumulation groups per half
            ps_o = psum.tile([128, C], fp32, tag="ps_qgtB", bufs=1)
            o_sb = work.tile([128, C], fp16)
            for h in range(2):
                c0, c1 = h * 256, (h + 1) * 256
                with at(10.85 + 0.7 * h):
                    for jc in range(JC):
                        nc.tensor.matmul(
                            ps_o[:, c0:c1],
                            lhsT=a_sb[:, jc, :],
                            rhs=v_sb[:, jc, c0:c1],
                            start=(jc == 0),
                            stop=False,
                        )
                    nc.tensor.matmul(
                        ps_o[:, c0:c1], lhsT=ones1, rhs=bv_row[:, c0:c1],
                        start=False, stop=True)
            with at(11.75):
                nc.vector.tensor_copy(out=o_sb[:, 0:256], in_=ps_o[:, 0:256])
                nc.sync.dma_start(out=out_d[:, 0:256], in_=o_sb[:, 0:256])
            with at(12.45):
                nc.scalar.activation(
                    out=o_sb[:, 256:512], in_=ps_o[:, 256:512], func=COPY)
                nc.scalar.dma_start(
                    out=out_d[:, 256:512], in_=o_sb[:, 256:512])

            if dbg_d is not None:
                z_dump = work.tile([128, JC, RCH], mybir.dt.float32)
                nc.vector.tensor_copy(out=z_dump, in_=ps_z)
                with at(12.5):
                    nc.sync.dma_start(out=dbg_d["dbg_qgt"][:, :],
                                      in_=qgt_sb.rearrange("p k r -> p (k r)"))
                    nc.sync.dma_start(out=dbg_d["dbg_et"][:, :],
                                      in_=et_sb.rearrange("p k r -> p (k r)"))
                    nc.sync.dma_start(out=dbg_d["dbg_z"][:, :],
                                      in_=z_dump.rearrange("p k r -> p (k r)"))
                    nc.sync.dma_start(out=dbg_d["dbg_w"][:, :],
                                      in_=w_sb.rearrange("p k r -> p (k r)"))
                    nc.sync.dma_start(out=dbg_d["dbg_a"][:, :],
                                      in_=a_sb.rearrange("p k r -> p (k r)"))
                    nc.sync.dma_start(out=dbg_d["dbg_v"][:, :],
                                      in_=v_sb.rearrange("p k r -> p (k r)"))
                    nc.sync.dma_start(out=dbg_d["dbg_cmt"][:, :],
                                      in_=cmt_sb.rearrange("p k r -> p (k r)"))
                    nc.sync.dma_start(out=dbg_d["dbg_b16"][:, :],
                                      in_=b16[:, :])


def _pack128(arr):
    """[n*128, f] row-chunked -> [128, n*f] (chunk-major along free axis)."""
    n = arr.shape[0] // 128
    return np.ascontiguousarray(
        arr.reshape(n, 128, -1).transpose(1, 0, 2).reshape(128, -1)
    )


def _host_prep(image_features, Wq, bq, Wk, bk, Wv, bv, sample_idx):
    """Build the 8 per-core input blobs (pure index/layout work plus
    weight-only constant folding)."""
    x = np.asarray(image_features, np.float32)
    sample_idx = np.asarray(sample_idx)
    Wq = np.asarray(Wq, np.float32)
    Wk = np.asarray(Wk, np.float32)
    Wv = np.asarray(Wv, np.float32)
    bq = np.asarray(bq, np.float32)
    bv = np.asarray(bv, np.float32)

    # score-collapse: S = x G x^T + ones (x) g   (bk drops: row shift)
    G = (Wq.T @ Wk).astype(np.float16)
    g = (bq @ Wk).astype(np.float32)

    # per-tile multiplicities -> banded count matrix Cm[j, t] = m_t[j - t]
    mod = (sample_idx % W).astype(np.int64)                  # [T, S]
    m = np.zeros((T, W), np.float32)
    np.add.at(m, (np.arange(T)[:, None], mod), 1.0)
    m += 1.0
    Cm = np.zeros((N, N), np.float32)
    rows = np.arange(T)
    for w in range(W):
        Cm[rows + w, rows] = m[:, w]

    pos = np.arange(N)
    counts = (np.minimum(pos, N - W) - np.maximum(pos - W + 1, 0) + 1)

    # padded versions for uniform band slicing
    XTp = np.zeros((B, C, N + 2 * 64), np.float16)
    for b in range(B):
        XTp[b, :, 64:64 + N] = x[b].T.astype(np.float16)
    Cmp = np.zeros((N + 2 * 64, N + 2 * 64), np.float32)
    Cmp[64:64 + N, 64:64 + N] = Cm

    g_p = _pack128(G.astype(np.float16))                     # [128, 4*512]
    wvt_p = _pack128(Wv.T.astype(np.float16))                # [128, 4*512]
    # g as per-partition fp32 columns: gcol[p, m] = g[m*128 + p]
    gcols = np.ascontiguousarray(g.reshape(KC, 128).T.astype(np.float32))

    misc = np.zeros((1, F_MISC), np.float16)
    misc[0, MISC_BV:MISC_BV + C] = bv
    misc[0, MISC_ONES:MISC_ONES + 128] = 1.0
    misc[0, MISC_G:MISC_G + C] = g.astype(np.float16)

    in_maps = []
    for core in range(NCORES):
        b, rc = divmod(core, NCORES // B)
        r0 = rc * RCH
        xt = XTp[b, :, r0:r0 + BAND]
        cm = np.ascontiguousarray(Cmp[r0:r0 + BAND, r0:r0 + BAND])
        # all-zero columns (padded t) would give Z=0 -> inf*0 = NaN on
        # device; a diagonal 1 keeps Z finite there and is masked out of W
        zero_cols = ~cm.any(axis=0)
        cm[zero_cols, zero_cols] = 1.0
        tl = np.arange(BAND)
        rl = np.arange(RCH)
        tg = r0 - 64 + tl
        rg = r0 + rl
        d = rg[None, :] - tg[:, None]
        valid = (d >= 0) & (d <= W - 1) & (tg[:, None] >= 0) & (tg[:, None] <= T - 1)
        maskw = np.where(
            valid, 1.0 / counts[rg][None, :], 0.0
        ).astype(np.float16)

        b16 = np.zeros((128, F16), np.float16)
        b16[:, OFF_XT:OFF_XT + KC * BAND] = _pack128(xt)
        for k, off in enumerate((OFF_G0, OFF_G1, OFF_G2, OFF_G3)):
            b16[:, off:off + C] = g_p[:, k * C:(k + 1) * C]
        for k, off in enumerate((OFF_WV0, OFF_WV1, OFF_WV2, OFF_WV3)):
            b16[:, off:off + C] = wvt_p[:, k * C:(k + 1) * C]
        b16[:, OFF_MW:OFF_MW + JC * RCH] = _pack128(maskw)
        # bf16/fp32-bit segments written through a uint16 view
        b16v = b16.view(np.uint16)
        b16v[:, OFF_GCOL:OFF_GCOL + 8] = gcols.view(np.uint16)
        b16v[:, OFF_CM:OFF_CM + JC * BAND] = _pack128(
            cm.astype(ml_dtypes.bfloat16)).view(np.uint16)
        b16v[:, OFF_IDB:OFF_IDB + 128] = np.eye(
            128, dtype=ml_dtypes.bfloat16).view(np.uint16)

        in_maps.append({"blob16": b16, "misc": misc})
    return in_maps


def run_on_cores(in_maps, trace=False, trace_cores=None):
    from concourse.bass_utils import run_bass_kernel_spmd

    if "nc" not in _CACHE:
        _CACHE["nc"] = _build_program()
    nc = _CACHE["nc"]
    return run_bass_kernel_spmd(
        nc, in_maps, list(range(NCORES)), trace=trace,
        trace_cores=(trace_cores or [0]) if trace else None,
    )


def kernel(image_features, Wq, bq, Wk, bk, Wv, bv, sample_idx):
    in_maps = _host_prep(image_features, Wq, bq, Wk, bk, Wv, bv, sample_idx)
    res = run_on_cores(in_maps, trace=False)
    out = np.empty((B, N, C), np.float32)
    for core in range(NCORES):
        b, rc = divmod(core, NCORES // B)
        out[b, rc * RCH:(rc + 1) * RCH, :] = (
            res.results[core]["out"].astype(np.float32))
    return out


# revision 38
# speedup vs baseline: 1.1232x; 1.1232x over previous
"""Trainium2 Bass kernel for ConsistentSelfAttentionTile.

Reference semantics: T=449 overlapping 64-token tiles; each tile attends to
352 KV tokens = 288 sampled (from a 9x replication of the tile) + the tile
itself; outputs overlap-add, then divide by overlap counts.

Algebraic collapse (verified vs the jax reference):
  * rep[:, idx, :] == tile[:, idx % 64, :], so the sampled KV tokens are tile
    rows with integer multiplicities m_t[w] = 1 + #{s : idx[t,s] % 64 == w}.
  * Per-tile Q/K/V are slices of the full-sequence projections, so all
    per-tile 64x64 score blocks are diagonal blocks of one banded 512x512
    score matrix S (band |i-j| <= 63).
  * S itself collapses: S = x G x^T + ones (x) g with G = Wq^T Wk and
    g = bq Wk, both host-precomputed (weight-only folding). Wk/Wq never ship
    to the device and the K projection disappears. bk only shifts rows
    (softmax-invariant): drop.
  * The device computes the TRANSPOSED projection directly:
        QG^T[c',r] = sum_c G[c,c'] x^T[c,r]  (+ g[c'] per-partition bias)
    via 16 128-col matmuls with G row-chunks as lhsT, so no PE transpose of
    QG and only ONE PSUM->SBUF hop on the score path. The g bias is folded
    in during that hop (Identity activation with a per-partition bias AP).
  * ST = x_band QG^T gives exp(ST) = E^T directly; no E transpose needed.
  * No rowmax subtraction: |S| <= ~50, inside fp32/bf16 exponent range; a
    row-constant shift cancels exactly in E/Z, so dropping the max is exact.
  * With E = exp(S), Cm[j,t] = m_t[j-t] (banded):
        Z = Cm^T E^T;  W = maskw * (1/Z);  U = Cm W;  out = (E^T o U)^T V
    maskw bakes in the valid-tile mask and the 1/counts overlap division.
  * bv is folded into the output PSUM as a rank-1 ones (x) bv matmul.
  * Cm^T is transposed on device (PE transpose) instead of shipped.

Sharding: 8 cores = 2 batches x 4 row-chunks of 128 output rows. Each core
computes its 128 rows end-to-end from a 256-column band of the input (no
cross-core communication).

Schedule: the input streams as 9 time-staggered pieces over the two HWDGE
queues (+ gpsimd SWDGE for one Wv chunk), ordered so the score chain
(x^T, G) lands first, cm/mask next, Wv last; consumers chase the stream.
512-col matmuls are SBUF-read-bound (~0.83ns/col) while 128-col chains
sustain ~2x that, so V and the output are emitted as 128-col pieces.
tile_wait_until hints pin the Tile scheduler to estimated arrival times.
The output is computed and evacuated in column halves over both queues.
"""

import os
import sys

import numpy as np

try:
    import ml_dtypes
except ImportError:
    ml_dtypes = None

for _p in ("/opt/trn_rl_repo",):
    if _p not in sys.path and os.path.isdir(_p):
        sys.path.insert(0, _p)

B, N, C, W = 2, 512, 512, 64
T = N - W + 1          # 449 tiles
RCH = 128              # output rows per core
NCORES = 8
BAND = 256             # per-core j/t band width (columns [r0-64, r0+192))
KC = C // 128          # 4 contraction chunks
JC = BAND // 128       # 2 band chunks

# blob16 layout (2-byte elements per partition), grouped by DMA piece.
# B queue (scalar/Act): XT | G2 | G3 | WV2
# A queue (sync/SP):    G0 | G1 | WV0 | CMBLOB (gcols idb cm mw) | WV1
# gpsimd SWDGE:         WV3
OFF_XT = 0                       # [128, 4, 256] fp16  x^T band chunks
OFF_G2 = OFF_XT + KC * BAND      # [128, 512] fp16   G row-chunk 2
OFF_G3 = OFF_G2 + C              # [128, 512]      (B piece 1 = XT|G2|G3)
OFF_G0 = OFF_G3 + C              # [128, 512]      (A piece 1 = G0|G1)
OFF_G1 = OFF_G0 + C              # [128, 512]
OFF_GCOL = OFF_G1 + C            # [128, 8] fp32 bits (A piece 2 = cm blob)
OFF_IDB = OFF_GCOL + 8           # [128, 128] bf16 identity (bitcast)
OFF_CM = OFF_IDB + 128           # [128, 2, 256] bf16 (count ints: exact)
OFF_MW = OFF_CM + JC * BAND      # [128, 2, 128] fp16 mask/counts
OFF_WV0 = OFF_MW + JC * RCH      # [128, 512]      (A piece 3 = WV0|WV1)
OFF_WV1 = OFF_WV0 + C            # [128, 512]
OFF_WV2 = OFF_WV1 + C            # [128, 512]      (pool piece = WV2|WV3)
OFF_WV3 = OFF_WV2 + C            # [128, 512]
F16 = OFF_WV3 + C

# misc row blob [1, 1152] fp16: bv | ones | g
MISC_BV = 0
MISC_ONES = C
MISC_G = C + 128
F_MISC = 2 * C + 128

N_WARM = 5             # dummy matmuls to ungate the PE clock
DEBUG_DUMPS = False    # dump intermediates to DRAM (debug builds only)

_CACHE = {}


def _slim_drain_and_barrier(self, tick_clock, wait_clock):
    """Cheaper TileContext exit: drain only the DMA-queue clocks, one
    sem-only barrier, then reset the Tile semaphores."""
    from concourse.vector_clock import ScopedClock, VectorClock
    from concourse.tile_scheduler import dmasw_start_idx, N_PROCS

    g = tick_clock.global_clock
    dma_clock = VectorClock()
    for idx in range(dmasw_start_idx, N_PROCS):
        t = g.peek_next(idx) - 1
        if t > 0:
            dma_clock.require_at_least(idx, t)
    drain_inst = self.nc.sync.drain()
    wait_clock.add_sem_waits(drain_inst.ins, ScopedClock({None: dma_clock}))
    self.nc.all_engine_barrier(sem_only=True)
    popped = self.nc._tile_sem_poison_stack.pop()
    assert popped is self._sem_poison
    self.nc.clear_and_free_semaphores(list(self.sems.allocated().values()))


def _build_program():
    import concourse.bacc as bacc
    import concourse.mybir as mybir
    import concourse.tile as tile

    fp16 = mybir.dt.float16
    # Skip Bass's preamble all-engine barrier: all real cross-engine deps
    # here are Tile semaphores, and the preamble const APs we read are
    # memset long before their consumers fire.
    orig_aeb = bacc.Bacc.all_engine_barrier

    def _noop_aeb(self, *, sem_only=False):
        return None

    bacc.Bacc.all_engine_barrier = _noop_aeb
    try:
        nc = bacc.Bacc("TRN2", target_bir_lowering=False, debug=False)
    finally:
        bacc.Bacc.all_engine_barrier = orig_aeb

    b16_d = nc.declare_dram_parameter("blob16", [128, F16], fp16, isOutput=False)
    misc_d = nc.declare_dram_parameter("misc", [1, F_MISC], fp16, isOutput=False)
    # output ships as fp16 (host upcasts): halves the final DMA + drain
    out_d = nc.declare_dram_parameter("out", [RCH, C], fp16, isOutput=True)
    dbg_d = None
    if DEBUG_DUMPS:
        bf16 = mybir.dt.bfloat16
        fp32 = mybir.dt.float32
        dbg_d = {
            "dbg_qgt": nc.declare_dram_parameter(
                "dbg_qgt", [128, KC * RCH], fp16, isOutput=True),
            "dbg_et": nc.declare_dram_parameter(
                "dbg_et", [128, JC * RCH], bf16, isOutput=True),
            "dbg_z": nc.declare_dram_parameter(
                "dbg_z", [128, JC * RCH], fp32, isOutput=True),
            "dbg_w": nc.declare_dram_parameter(
                "dbg_w", [128, JC * RCH], bf16, isOutput=True),
            "dbg_a": nc.declare_dram_parameter(
                "dbg_a", [128, JC * RCH], fp16, isOutput=True),
            "dbg_v": nc.declare_dram_parameter(
                "dbg_v", [128, JC * C], fp16, isOutput=True),
            "dbg_cmt": nc.declare_dram_parameter(
                "dbg_cmt", [128, JC * BAND], bf16, isOutput=True),
            "dbg_b16": nc.declare_dram_parameter(
                "dbg_b16", [128, F16], fp16, isOutput=True),
        }

    orig_dab = tile.TileContext._drain_and_barrier
    tile.TileContext._drain_and_barrier = _slim_drain_and_barrier
    try:
        _emit_body(nc, tile, mybir, b16_d, misc_d, out_d, dbg_d)
    finally:
        tile.TileContext._drain_and_barrier = orig_dab

    nc.compile()
    return nc


def _emit_body(nc, tile, mybir, b16_d, misc_d, out_d, dbg_d=None):
    fp32 = mybir.dt.float32
    fp16 = mybir.dt.float16
    bf16 = mybir.dt.bfloat16
    COPY = mybir.ActivationFunctionType.Copy
    IDENT = mybir.ActivationFunctionType.Identity

    with tile.TileContext(nc) as tc:

        def at(us):
            """Schedule hint: don't start the instructions in this block
            before `us` microseconds (relative to kernel-body start)."""
            return tc.tile_wait_until(us / 1000.0)

        with (
            tc.tile_pool(name="consts", bufs=1) as consts,
            tc.tile_pool(name="work", bufs=1) as work,
            tc.tile_pool(name="psum", bufs=1, space="PSUM") as psum,
        ):
            b16 = consts.tile([128, F16], fp16)
            misc = consts.tile([1, F_MISC], fp16)
            warm = work.tile([128, 512], bf16)

            # PE warm-up operand: zeros so nothing downstream can see junk.
            nc.gpsimd.memset(warm[:, :], 0)
            nc.gpsimd.dma_start(out=misc[:, :], in_=misc_d[:, :])

            # ---- input DMA: 2 HWDGE queues x 2 chained pieces + pool ----
            from concourse.tile_rust import add_dep_helper

            def piece(eng, c0, c1, after=None):
                h = eng.dma_start(out=b16[:, c0:c1], in_=b16_d[:, c0:c1])
                if after is not None:
                    add_dep_helper(h.ins, after.ins, True, "input DMA chain")
                return h

            # B (scalar): score stream, then wv2|wv3.  A (sync): G0|G1,
            # gcol/idb/cm/mw, then wv0|wv1.  Wave 2 chains on wave 1 --
            # concurrent pieces interleave across the shared SDMA pool, so
            # at most one piece per queue is ever in flight.
            b1 = piece(nc.scalar, OFF_XT, OFF_G0)            # XT|G2|G3
            a1 = piece(nc.sync, OFF_G0, OFF_WV0)             # G0|G1|cmblob
            b2 = piece(nc.scalar, OFF_WV2, F16, after=b1)    # WV2|WV3
            a2 = piece(nc.sync, OFF_WV0, OFF_WV2, after=a1)  # WV0|WV1

            # ---- SBUF views ----
            xt_sb = b16[:, OFF_XT:OFF_XT + KC * BAND].rearrange(
                "p (k j) -> p k j", k=KC)
            g_chunk = [b16[:, OFF_G0:OFF_G0 + C], b16[:, OFF_G1:OFF_G1 + C],
                       b16[:, OFF_G2:OFF_G2 + C], b16[:, OFF_G3:OFF_G3 + C]]
            wv_chunk = [b16[:, OFF_WV0:OFF_WV0 + C], b16[:, OFF_WV1:OFF_WV1 + C],
                        b16[:, OFF_WV2:OFF_WV2 + C], b16[:, OFF_WV3:OFF_WV3 + C]]
            gcol = b16[:, OFF_GCOL:OFF_GCOL + 8].bitcast(fp32)
            idb = b16[:, OFF_IDB:OFF_IDB + 128].bitcast(bf16)
            cm_sb = b16[:, OFF_CM:OFF_CM + JC * BAND].bitcast(bf16).rearrange(
                "p (k t) -> p k t", k=JC)
            mw_sb = b16[:, OFF_MW:OFF_MW + JC * RCH].rearrange(
                "p (k r) -> p k r", k=JC)
            bv_row = misc[0:1, MISC_BV:MISC_BV + C]
            ones1 = misc[0:1, MISC_ONES:MISC_ONES + 128]
            g_row = misc[0:1, MISC_G:MISC_G + C]

            # ---- PE clock-gate warm-up (runs while the DMA lands) ----
            ps_aux = psum.tile([128, 512], fp32, tag="ps_aux", bufs=1)
            for _ in range(N_WARM):
                nc.tensor.matmul(
                    ps_aux, lhsT=warm[:, 0:128], rhs=warm[:, :],
                    start=True, stop=True,
                )

            # ---- V[jc][j 128, c 512] = x Wv, 512-col matmuls chasing the
            # Wv stream (wv2|wv3 ride the pool queue and land first)
            v_sb = work.tile([128, JC, C], fp16)
            ps_v0 = psum.tile([128, C], fp32, tag="ps_v0", bufs=1)
            ps_v1 = psum.tile([128, C], fp32, tag="ps_v1", bufs=1)
            ps_vjc = (ps_v0, ps_v1)
            wv_at = {0: 6.9, 1: 7.95, 2: 8.65, 3: 9.35}

            def v_block(cc, start, stop):
                with at(wv_at[cc]):
                    for jc in range(JC):
                        nc.tensor.matmul(
                            ps_vjc[jc],
                            lhsT=xt_sb[:, cc, jc * 128:(jc + 1) * 128],
                            rhs=wv_chunk[cc],
                            start=start,
                            stop=stop,
                        )

            # ---- QG^T[c',r] = sum_cc G[cc,c'] xt[cc,r] + g[c'] (x) ones ----
            # PSUM banks allow only ONE open accumulation group each, so the
            # 4 c'-tiles run as two bank-pairs: (m0 in bank A, m2 in bank B)
            # chunk-chase the G stream, then (m1, m3) reread the resident
            # chunks. The g bias rank-1 closes each group.
            ps_qgtA = psum.tile([128, C], fp32, tag="ps_qgtA", bufs=1)
            ps_qgtB = psum.tile([128, C], fp32, tag="ps_qgtB", bufs=1)
            # pass 2 lands in the (not-yet-used) ST and Z banks so it does
            # not WAR-wait on pass 1's PSUM->SBUF copies
            ps_qgt2a = psum.tile([128, JC, RCH], fp32, tag="ps_st", bufs=1)
            ps_qgt2b = psum.tile([128, JC, RCH], fp32, tag="ps_z", bufs=1)
            ps_of = {0: ps_qgtA[:, 0:RCH], 2: ps_qgtB[:, 0:RCH],
                     1: ps_qgt2a[:, 0, :], 3: ps_qgt2b[:, 0, :]}
            qgt_sb = work.tile([128, KC, RCH], fp16)
            for pi, pair in enumerate(((0, 2), (1, 3))):
                with at(4.3 + 0.85 * pi):
                    for i, cc in enumerate((0, 1, 2, 3)):
                        for m in pair:
                            nc.tensor.matmul(
                                ps_of[m],
                                lhsT=g_chunk[cc][:, m * 128:(m + 1) * 128],
                                rhs=xt_sb[:, cc, 64:64 + RCH],
                                start=(i == 0),
                                stop=(i == KC - 1),
                            )
                # PSUM->SBUF with the g bias folded as a per-partition scalar
                with at(5.2 + 0.85 * pi):
                    for m in pair:
                        nc.vector.tensor_scalar_add(
                            qgt_sb[:, m, :], ps_of[m], gcol[:, m:m + 1])

            # ---- transposed scores ST[j 256, r 128] and E^T = exp(ST) ----
            ps_st = psum.tile([128, JC, RCH], fp32, tag="ps_st", bufs=1)
            with at(6.3):
                for jc in range(JC):
                    for k in range(KC):
                        nc.tensor.matmul(
                            ps_st[:, jc, :],
                            lhsT=xt_sb[:, k, jc * 128:(jc + 1) * 128],
                            rhs=qgt_sb[:, k, :],
                            start=(k == 0),
                            stop=(k == KC - 1),
                        )
            et_sb = work.tile([128, JC, RCH], bf16)
            with at(7.0):
                nc.scalar.activation(
                    out=et_sb[:, 0, :], in_=ps_st[:, 0, :],
                    func=mybir.ActivationFunctionType.Exp,
                    bias=0.0, scale=1.0,
                )
            with at(7.25):
                nc.scalar.activation(
                    out=et_sb[:, 1, :], in_=ps_st[:, 1, :],
                    func=mybir.ActivationFunctionType.Exp,
                    bias=0.0, scale=1.0,
                )

            # Cm^T on device: 4 PE transposes in the PE gap while exps run
            ps_cmt = psum.tile([128, JC, BAND], bf16, tag="ps_aux", bufs=1)
            with at(7.0):
                for tch in range(JC):
                    for jc in range(JC):
                        nc.tensor.transpose(
                            ps_cmt[:, tch, jc * 128:(jc + 1) * 128],
                            cm_sb[:, jc, tch * 128:(tch + 1) * 128],
                            idb,
                        )
            cmt_sb = work.tile([128, JC, BAND], bf16)
            with at(7.45):
                nc.vector.tensor_copy(out=cmt_sb, in_=ps_cmt)

            v_block(0, True, False)     # wv0|wv1 land with wave 2

            # Z[t 128, r 128] per tch = sum_j Cm[j,t] E^T[j,r]
            ps_z = psum.tile([128, JC, RCH], fp32, tag="ps_z", bufs=1)
            with at(7.7):
                for tch in range(JC):
                    for jc in range(JC):
                        nc.tensor.matmul(
                            ps_z[:, tch, :],
                            lhsT=cm_sb[:, jc, tch * 128:(tch + 1) * 128],
                            rhs=et_sb[:, jc, :],
                            start=(jc == 0),
                            stop=(jc == JC - 1),
                        )

            # W[t, r] = maskw / Z: reciprocal on Vector, mask-mul on Pool
            rz_sb = work.tile([128, JC, RCH], fp32)
            w_sb = work.tile([128, JC, RCH], bf16)
            with at(8.15):
                nc.vector.reciprocal_approx_fast(out=rz_sb, in_=ps_z)
            with at(8.6):
                nc.vector.tensor_mul(w_sb, rz_sb, mw_sb)

            v_block(1, False, False)
            v_block(2, False, False)
            v_block(3, False, True)

            # U[j 128, r 128] per jc = sum_t Cm^T[t,j] W[t,r]
            ps_u = psum.tile([128, JC, RCH], fp32, tag="ps_qgtA", bufs=1)
            with at(10.1):
                for jc in range(JC):
                    for tch in range(JC):
                        nc.tensor.matmul(
                            ps_u[:, jc, :],
                            lhsT=cmt_sb[:, tch, jc * 128:(jc + 1) * 128],
                            rhs=w_sb[:, tch, :],
                            start=(tch == 0),
                            stop=(tch == JC - 1),
                        )
            a_sb = work.tile([128, JC, RCH], fp16)
            with at(10.4):
                nc.vector.tensor_mul(a_sb, ps_u, et_sb)

            # V PSUM -> SBUF in column halves: h0 on scalar, h1 on vector
            with at(10.15):
                nc.scalar.activation(out=v_sb[:, 0, 0:256],
                                     in_=ps_v0[:, 0:256], func=COPY)
                nc.scalar.activation(out=v_sb[:, 1, 0:256],
                                     in_=ps_v1[:, 0:256], func=COPY)
            with at(10.85):
                nc.vector.tensor_copy(out=v_sb[:, 0, 256:512],
                                      in_=ps_v0[:, 256:512])
                nc.vector.tensor_copy(out=v_sb[:, 1, 256:512],
                                      in_=ps_v1[:, 256:512])

            # out[r 128, c 512] = sum_j A[j,r] V[j,c] + ones (x) bv,
            # 256-col halves, sequential accumulation groups per half
            ps_o = psum.tile([128, C], fp32, tag="ps_qgtB", bufs=1)
            o_sb = work.tile([128, C], fp16)
            for h in range(2):
                c0, c1 = h * 256, (h + 1) * 256
                with at(11.1 + 0.7 * h):
                    for jc in range(JC):
                        nc.tensor.matmul(
                            ps_o[:, c0:c1],
                            lhsT=a_sb[:, jc, :],
                            rhs=v_sb[:, jc, c0:c1],
                            start=(jc == 0),
                            stop=False,
                        )
                    nc.tensor.matmul(
                        ps_o[:, c0:c1], lhsT=ones1, rhs=bv_row[:, c0:c1],
                        start=False, stop=True)
            with at(12.05):
                nc.vector.tensor_copy(out=o_sb[:, 0:256], in_=ps_o[:, 0:256])
                nc.sync.dma_start(out=out_d[:, 0:256], in_=o_sb[:, 0:256])
            with at(12.75):
                nc.scalar.activation(
                    out=o_sb[:, 256:512], in_=ps_o[:, 256:512], func=COPY)
                nc.scalar.dma_start(
                    out=out_d[:, 256:512], in_=o_sb[:, 256:512])

            if dbg_d is not None:
                z_dump = work.tile([128, JC, RCH], mybir.dt.float32)
                nc.vector.tensor_copy(out=z_dump, in_=ps_z)
                with at(12.5):
                    nc.sync.dma_start(out=dbg_d["dbg_qgt"][:, :],
                                      in_=qgt_sb.rearrange("p k r -> p (k r)"))
                    nc.sync.dma_start(out=dbg_d["dbg_et"][:, :],
                                      in_=et_sb.rearrange("p k r -> p (k r)"))
                    nc.sync.dma_start(out=dbg_d["dbg_z"][:, :],
                                      in_=z_dump.rearrange("p k r -> p (k r)"))
                    nc.sync.dma_start(out=dbg_d["dbg_w"][:, :],
                                      in_=w_sb.rearrange("p k r -> p (k r)"))
                    nc.sync.dma_start(out=dbg_d["dbg_a"][:, :],
                                      in_=a_sb.rearrange("p k r -> p (k r)"))
                    nc.sync.dma_start(out=dbg_d["dbg_v"][:, :],
                                      in_=v_sb.rearrange("p k r -> p (k r)"))
                    nc.sync.dma_start(out=dbg_d["dbg_cmt"][:, :],
                                      in_=cmt_sb.rearrange("p k r -> p (k r)"))
                    nc.sync.dma_start(out=dbg_d["dbg_b16"][:, :],
                                      in_=b16[:, :])


def _pack128(arr):
    """[n*128, f] row-chunked -> [128, n*f] (chunk-major along free axis)."""
    n = arr.shape[0] // 128
    return np.ascontiguousarray(
        arr.reshape(n, 128, -1).transpose(1, 0, 2).reshape(128, -1)
    )


def _host_prep(image_features, Wq, bq, Wk, bk, Wv, bv, sample_idx):
    """Build the 8 per-core input blobs (pure index/layout work plus
    weight-only constant folding)."""
    x = np.asarray(image_features, np.float32)
    sample_idx = np.asarray(sample_idx)
    Wq = np.asarray(Wq, np.float32)
    Wk = np.asarray(Wk, np.float32)
    Wv = np.asarray(Wv, np.float32)
    bq = np.asarray(bq, np.float32)
    bv = np.asarray(bv, np.float32)

    # score-collapse: S = x G x^T + ones (x) g   (bk drops: row shift)
    G = (Wq.T @ Wk).astype(np.float16)
    g = (bq @ Wk).astype(np.float32)

    # per-tile multiplicities -> banded count matrix Cm[j, t] = m_t[j - t]
    mod = (sample_idx % W).astype(np.int64)                  # [T, S]
    m = np.zeros((T, W), np.float32)
    np.add.at(m, (np.arange(T)[:, None], mod), 1.0)
    m += 1.0
    Cm = np.zeros((N, N), np.float32)
    rows = np.arange(T)
    for w in range(W):
        Cm[rows + w, rows] = m[:, w]

    pos = np.arange(N)
    counts = (np.minimum(pos, N - W) - np.maximum(pos - W + 1, 0) + 1)

    # padded versions for uniform band slicing
    XTp = np.zeros((B, C, N + 2 * 64), np.float16)
    for b in range(B):
        XTp[b, :, 64:64 + N] = x[b].T.astype(np.float16)
    Cmp = np.zeros((N + 2 * 64, N + 2 * 64), np.float32)
    Cmp[64:64 + N, 64:64 + N] = Cm

    g_p = _pack128(G.astype(np.float16))                     # [128, 4*512]
    wvt_p = _pack128(Wv.T.astype(np.float16))                # [128, 4*512]
    # g as per-partition fp32 columns: gcol[p, m] = g[m*128 + p]
    gcols = np.ascontiguousarray(g.reshape(KC, 128).T.astype(np.float32))

    misc = np.zeros((1, F_MISC), np.float16)
    misc[0, MISC_BV:MISC_BV + C] = bv
    misc[0, MISC_ONES:MISC_ONES + 128] = 1.0
    misc[0, MISC_G:MISC_G + C] = g.astype(np.float16)

    in_maps = []
    for core in range(NCORES):
        b, rc = divmod(core, NCORES // B)
        r0 = rc * RCH
        xt = XTp[b, :, r0:r0 + BAND]
        cm = np.ascontiguousarray(Cmp[r0:r0 + BAND, r0:r0 + BAND])
        # all-zero columns (padded t) would give Z=0 -> inf*0 = NaN on
        # device; a diagonal 1 keeps Z finite there and is masked out of W
        zero_cols = ~cm.any(axis=0)
        cm[zero_cols, zero_cols] = 1.0
        tl = np.arange(BAND)
        rl = np.arange(RCH)
        tg = r0 - 64 + tl
        rg = r0 + rl
        d = rg[None, :] - tg[:, None]
        valid = (d >= 0) & (d <= W - 1) & (tg[:, None] >= 0) & (tg[:, None] <= T - 1)
        maskw = np.where(
            valid, 1.0 / counts[rg][None, :], 0.0
        ).astype(np.float16)

        b16 = np.zeros((128, F16), np.float16)
        b16[:, OFF_XT:OFF_XT + KC * BAND] = _pack128(xt)
        for k, off in enumerate((OFF_G0, OFF_G1, OFF_G2, OFF_G3)):
            b16[:, off:off + C] = g_p[:, k * C:(k + 1) * C]
        for k, off in enumerate((OFF_WV0, OFF_WV1, OFF_WV2, OFF_WV3)):
            b16[:, off:off + C] = wvt_p[:, k * C:(k + 1) * C]
        b16[:, OFF_MW:OFF_MW + JC * RCH] = _pack128(maskw)
        # bf16/fp32-bit segments written through a uint16 view
        b16v = b16.view(np.uint16)
        b16v[:, OFF_GCOL:OFF_GCOL + 8] = gcols.view(np.uint16)
        b16v[:, OFF_CM:OFF_CM + JC * BAND] = _pack128(
            cm.astype(ml_dtypes.bfloat16)).view(np.uint16)
        b16v[:, OFF_IDB:OFF_IDB + 128] = np.eye(
            128, dtype=ml_dtypes.bfloat16).view(np.uint16)

        in_maps.append({"blob16": b16, "misc": misc})
    return in_maps


def run_on_cores(in_maps, trace=False, trace_cores=None):
    from concourse.bass_utils import run_bass_kernel_spmd

    if "nc" not in _CACHE:
        _CACHE["nc"] = _build_program()
    nc = _CACHE["nc"]
    return run_bass_kernel_spmd(
        nc, in_maps, list(range(NCORES)), trace=trace,
        trace_cores=(trace_cores or [0]) if trace else None,
    )


def kernel(image_features, Wq, bq, Wk, bk, Wv, bv, sample_idx):
    in_maps = _host_prep(image_features, Wq, bq, Wk, bk, Wv, bv, sample_idx)
    res = run_on_cores(in_maps, trace=False)
    out = np.empty((B, N, C), np.float32)
    for core in range(NCORES):
        b, rc = divmod(core, NCORES // B)
        out[b, rc * RCH:(rc + 1) * RCH, :] = (
            res.results[core]["out"].astype(np.float32))
    return out
